# revision 14
# baseline (speedup 1.0000x reference)
"""Trainium2 Bass kernel for nn_CrossAttention (linear-attention block).

Math (per batch b):
    q = x @ Wq; k = x @ Wk; v = x @ Wv
    q_sm = softmax(q, axis=-1) * D^-0.5          (feature softmax)
    k_sm = softmax(k, axis=-2)                   (sequence softmax)
    ctx  = k_sm^T @ v                            [D, D]
    out  = (q_sm @ ctx) @ Wo + bo
    y    = layernorm(x + out) * gamma + beta

Sharding: data-parallel, one batch per NeuronCore (B == 8 == n_cores).

Device algorithm (no softmax max-subtraction needed: q,k ~ N(0,1)):
    ek = exp(k);  S_k[d] = sum_l ek[l,d]
    CT[e,d]   = sum_l v[l,e] * ek[l,d]           (ctx^T, unnormalized)
    C2[d,e']  = (CT^T @ Wo)[d,e'] * D^-0.5 / S_k[d]
    eq = exp(q);  S_q[l] = sum_d eq[l,d]
    out[l,e'] = (eq @ C2)[l,e'] / S_q[l]
    y = LN(x + bo + out)
All matmuls in bf16 with fp32 PSUM accumulation. x^T (bf16) is the only
stationary source for the projections; q is produced transposed (qT) so
exp(qT) blocks serve directly as matmul lhsT for the attn GEMM.
"""

import os
import numpy as np
import ml_dtypes
from contextlib import ExitStack

_KP = os.environ.get("KP", "full")  # debug: A | C | C2 | QT | full

B, L, D = 8, 4096, 1024
P = 128
LT = L // P       # 32 row tiles
FC = D // P       # 8 feature chunks
NG = 8            # l-groups for the q phase
GL = L // NG      # 512 columns per l-group
LN_EPS = 1e-5

_CACHE = {}


def _build_nc():
    import concourse.tile as tile
    from concourse import bacc, mybir

    f32 = mybir.dt.float32
    bf16 = mybir.dt.bfloat16
    FT = mybir.ActivationFunctionType
    OP = mybir.AluOpType

    nc = bacc.Bacc("TRN2", target_bir_lowering=False, debug=False,
                   enable_asserts=False)

    xT = nc.dram_tensor("xT", [D, L], bf16, kind="ExternalInput").ap()
    xr = nc.dram_tensor("xr", [L, D], f32, kind="ExternalInput").ap()
    wk_d = nc.dram_tensor("wk", [D, D], bf16, kind="ExternalInput").ap()
    wv_d = nc.dram_tensor("wv", [D, D], bf16, kind="ExternalInput").ap()
    wq_d = nc.dram_tensor("wq", [D, D], bf16, kind="ExternalInput").ap()
    wo_d = nc.dram_tensor("wo", [D, D], bf16, kind="ExternalInput").ap()
    gam_d = nc.dram_tensor("gamma_bc", [P, D], f32, kind="ExternalInput").ap()
    bet_d = nc.dram_tensor("beta_bc", [P, D], f32, kind="ExternalInput").ap()
    ones_d = nc.dram_tensor("ones_col", [P, 1], bf16, kind="ExternalInput").ap()
    y_d = nc.dram_tensor("y", [L, D], f32, kind="ExternalOutput").ap()

    with tile.TileContext(nc) as tc, ExitStack() as ctx:
        wpool = ctx.enter_context(tc.tile_pool(name="w", bufs=2))
        ekp = ctx.enter_context(tc.tile_pool(name="ekp", bufs=32))
        sbp = ctx.enter_context(tc.tile_pool(name="sbp", bufs=2))
        eqp = ctx.enter_context(tc.tile_pool(name="eqp", bufs=8))
        xtgp = ctx.enter_context(tc.tile_pool(name="xtgp", bufs=10))
        ctp = ctx.enter_context(tc.tile_pool(name="ctp", bufs=8))
        c2p = ctx.enter_context(tc.tile_pool(name="c2p", bufs=8))
        cst = ctx.enter_context(tc.tile_pool(name="cst", bufs=1))
        vec = ctx.enter_context(tc.tile_pool(name="vec", bufs=2))
        ps_big = ctx.enter_context(tc.tile_pool(name="psb", bufs=2, space="PSUM"))
        ps_qt = ctx.enter_context(tc.tile_pool(name="psq", bufs=2, space="PSUM"))
        ps_sk = ctx.enter_context(tc.tile_pool(name="pssk", bufs=1, space="PSUM"))
        dram = ctx.enter_context(tc.tile_pool(name="dram", bufs=1, space="DRAM"))

        # ---- constants / weights -------------------------------------
        def load_w(dram_ap, name):
            t = wpool.tile([P, FC, D], bf16, tag="W", name=name)
            nc.sync.dma_start(t[:], dram_ap.rearrange("(fo fi) d -> fi fo d", fi=P))
            return t

        ones_sb = cst.tile([P, 1], bf16, tag="ones")
        nc.sync.dma_start(ones_sb[:], ones_d[:])
        eps_sb = cst.tile([P, 1], f32, tag="eps")
        nc.vector.memset(eps_sb[:], LN_EPS)
        gam_sb = cst.tile([P, D], f32, tag="gam")
        nc.sync.dma_start(gam_sb[:], gam_d[:])
        bet_sb = cst.tile([P, D], f32, tag="bet")
        nc.sync.dma_start(bet_sb[:], bet_d[:])

        wk = load_w(wk_d, "wk")
        wv = load_w(wv_d, "wv")

        v_spill = dram.tile([L, D], bf16)
        sk_dram = dram.tile([1, D], f32)

        xTr = xT.rearrange("(fo fi) l -> fi fo l", fi=P)

        # ---- phase A: k/v projections, exp(k), S_k, spill v ----------
        ek_tiles = []
        sk_ps = ps_sk.tile([1, D], f32, tag="skq")
        for i in range(LT):
            xt_i = sbp.tile([P, FC, P], bf16, tag="xta")
            nc.sync.dma_start(xt_i[:], xTr[:, :, i * P:(i + 1) * P])

            k_ps = ps_big.tile([P, D], f32, tag="big")
            for f in range(FC):
                for n0 in (0, 512):
                    nc.tensor.matmul(k_ps[:, n0:n0 + 512], xt_i[:, f, :],
                                     wk[:, f, n0:n0 + 512],
                                     start=(f == 0), stop=(f == FC - 1))
            ek_i = ekp.tile([P, D], bf16, tag="ek")
            nc.scalar.activation(ek_i[:], k_ps[:], FT.Exp)
            ek_tiles.append(ek_i)

            v_ps = ps_big.tile([P, D], f32, tag="big")
            for f in range(FC):
                for n0 in (0, 512):
                    nc.tensor.matmul(v_ps[:, n0:n0 + 512], xt_i[:, f, :],
                                     wv[:, f, n0:n0 + 512],
                                     start=(f == 0), stop=(f == FC - 1))
            v_st = sbp.tile([P, D], bf16, tag="vst")
            nc.vector.tensor_copy(v_st[:], v_ps[:])
            nc.sync.dma_start(v_spill[i * P:(i + 1) * P, :], v_st[:])

            # S_k[d] += sum_{l in tile} ek[l, d]
            for n0 in (0, 512):
                nc.tensor.matmul(sk_ps[:, n0:n0 + 512], ones_sb[:],
                                 ek_i[:, n0:n0 + 512],
                                 start=(i == 0), stop=(i == LT - 1))

        # S_k row -> [P, FC] column layout via DRAM round-trip
        sk_row = cst.tile([1, D], f32, tag="skrow")
        nc.vector.tensor_copy(sk_row[:], sk_ps[:])
        nc.sync.dma_start(sk_dram[:], sk_row[:])
        sk_col = cst.tile([P, FC], f32, tag="skcol")
        nc.sync.dma_start(sk_col[:], sk_dram[0, :].rearrange("(o p) -> p o", p=P))
        rsk = cst.tile([P, FC], f32, tag="rsk")
        nc.vector.reciprocal(rsk[:], sk_col[:])
        nc.vector.tensor_scalar_mul(rsk[:], rsk[:], float(D) ** -0.5)

        # ---- context: CT[e,d] = sum_l v[l,e] * ek[l,d] ---------------
        ct_tiles = []
        for e in range(FC):
            c_ps = ps_big.tile([P, D], f32, tag="big")
            for j in range(4):
                vblk = sbp.tile([P, 8, P], bf16, tag="vct")
                nc.sync.dma_start(
                    vblk[:],
                    v_spill[j * 1024:(j + 1) * 1024, e * P:(e + 1) * P]
                    .rearrange("(o p) c -> p o c", p=P))
                for l8 in range(8):
                    lc = j * 8 + l8
                    for n0 in (0, 512):
                        nc.tensor.matmul(c_ps[:, n0:n0 + 512], vblk[:, l8, :],
                                         ek_tiles[lc][:, n0:n0 + 512],
                                         start=(lc == 0), stop=(lc == LT - 1))
            ct_e = ctp.tile([P, D], bf16, tag="ct")
            nc.vector.tensor_copy(ct_e[:], c_ps[:])
            ct_tiles.append(ct_e)

        wq = load_w(wq_d, "wq")
        wo = load_w(wo_d, "wo")

        # ---- C2[d, e'] = (CT^T @ Wo) * rsk[d] ------------------------
        c2_tiles = []
        for d in range(FC):
            c2_ps = ps_big.tile([P, D], f32, tag="big")
            for ec in range(FC):
                for n0 in (0, 512):
                    nc.tensor.matmul(c2_ps[:, n0:n0 + 512],
                                     ct_tiles[ec][:, d * P:(d + 1) * P],
                                     wo[:, ec, n0:n0 + 512],
                                     start=(ec == 0), stop=(ec == FC - 1))
            c2_d = c2p.tile([P, D], bf16, tag="c2")
            nc.vector.tensor_scalar_mul(c2_d[:], c2_ps[:], rsk[:, d:d + 1])
            c2_tiles.append(c2_d)

        # ---- phase B: qT, exp, attn, residual + layernorm ------------
        for g in range(NG):
            xtg = [xtgp.tile([P, GL], bf16, tag="xtg", name=f"xtg_{g}_{f}")
                   for f in range(FC)]
            for f in range(FC):
                nc.sync.dma_start(xtg[f][:], xTr[:, f, g * GL:(g + 1) * GL])

            eq_g = []
            for d in range(FC):
                qt_ps = ps_qt.tile([P, GL], f32, tag="qt")
                for f in range(FC):
                    nc.tensor.matmul(qt_ps[:], wq[:, f, d * P:(d + 1) * P],
                                     xtg[f][:],
                                     start=(f == 0), stop=(f == FC - 1))
                eq_d = eqp.tile([P, GL], bf16, tag="eq")
                nc.scalar.activation(eq_d[:], qt_ps[:], FT.Exp)
                eq_g.append(eq_d)

            for t in range(GL // P):
                lt = g * (GL // P) + t
                ts = slice(t * P, (t + 1) * P)
                at_ps = ps_big.tile([P, D], f32, tag="big")
                for d in range(FC):
                    for n0 in (0, 512):
                        nc.tensor.matmul(at_ps[:, n0:n0 + 512],
                                         eq_g[d][:, ts],
                                         c2_tiles[d][:, n0:n0 + 512],
                                         start=(d == 0), stop=(d == FC - 1))
                sq_ps = ps_sk.tile([P, 1], f32, tag="skq", name=f"sq_{lt}")
                for d in range(FC):
                    nc.tensor.matmul(sq_ps[:], eq_g[d][:, ts], ones_sb[:],
                                     start=(d == 0), stop=(d == FC - 1))
                s_q = vec.tile([P, 1], f32, tag="s_q")
                nc.vector.reciprocal(s_q[:], sq_ps[:])

                xr_t = sbp.tile([P, D], f32, tag="xr")
                nc.sync.dma_start(xr_t[:], xr[lt * P:(lt + 1) * P, :])

                r_t = sbp.tile([P, D], f32, tag="r")
                rsum = vec.tile([P, 1], f32, tag="rsum")
                nc.vector.scalar_tensor_tensor(
                    r_t[:], at_ps[:], s_q[:], xr_t[:],
                    op0=OP.mult, op1=OP.add, accum_out=rsum[:])

                sq_scr = ps_big.tile([P, D], f32, tag="big")
                rsumsq = vec.tile([P, 1], f32, tag="rsumsq")
                nc.scalar.activation(sq_scr[:], r_t[:], FT.Square,
                                     accum_out=rsumsq[:])

                mu = vec.tile([P, 1], f32, tag="mu")
                nc.vector.tensor_scalar_mul(mu[:], rsum[:], 1.0 / D)
                musq = vec.tile([P, 1], f32, tag="musq")
                nc.vector.tensor_mul(musq[:], mu[:], mu[:])
                var = vec.tile([P, 1], f32, tag="var")
                nc.vector.scalar_tensor_tensor(
                    var[:], rsumsq[:], 1.0 / D, musq[:],
                    op0=OP.mult, op1=OP.subtract)
                std = vec.tile([P, 1], f32, tag="std")
                nc.scalar.activation(std[:], var[:], FT.Sqrt, bias=eps_sb[:])
                rstd = vec.tile([P, 1], f32, tag="rstd")
                nc.vector.reciprocal(rstd[:], std[:])

                nc.vector.scalar_tensor_tensor(
                    r_t[:], r_t[:], mu[:], gam_sb[:],
                    op0=OP.subtract, op1=OP.mult)
                nc.vector.scalar_tensor_tensor(
                    r_t[:], r_t[:], rstd[:], bet_sb[:],
                    op0=OP.mult, op1=OP.add)
                nc.sync.dma_start(y_d[lt * P:(lt + 1) * P, :], r_t[:])

    nc.compile()
    return nc


def _get_nc():
    if "nc" not in _CACHE:
        _CACHE["nc"] = _build_nc()
    return _CACHE["nc"]


def make_in_maps(x, Wq, Wk, Wv, Wo, bo, gamma, beta):
    bf = ml_dtypes.bfloat16
    x = np.asarray(x, dtype=np.float32)
    in_maps = []
    wk_b = np.asarray(Wk, np.float32).astype(bf)
    wv_b = np.asarray(Wv, np.float32).astype(bf)
    wq_b = np.asarray(Wq, np.float32).astype(bf)
    wo_b = np.asarray(Wo, np.float32).astype(bf)
    gam = np.ascontiguousarray(
        np.broadcast_to(np.asarray(gamma, np.float32), (P, D)))
    bet = np.ascontiguousarray(
        np.broadcast_to(np.asarray(beta, np.float32), (P, D)))
    ones = np.ones((P, 1), dtype=bf)
    bo32 = np.asarray(bo, np.float32)
    for b in range(B):
        in_maps.append({
            "xT": np.ascontiguousarray(x[b].T).astype(bf),
            "xr": x[b] + bo32,
            "wk": wk_b, "wv": wv_b, "wq": wq_b, "wo": wo_b,
            "gamma_bc": gam, "beta_bc": bet, "ones_col": ones,
        })
    return in_maps


def kernel(x, Wq, Wk, Wv, Wo, bo, gamma, beta):
    from concourse import bass_utils
    nc = _get_nc()
    in_maps = make_in_maps(x, Wq, Wk, Wv, Wo, bo, gamma, beta)
    res = bass_utils.run_bass_kernel_spmd(nc, in_maps, core_ids=list(range(B)))
    out = np.stack([res.results[b]["y"] for b in range(B)], axis=0)
    return out.astype(np.float32)


# revision 19
# speedup vs baseline: 126.4279x; 126.4279x over previous
"""Trainium2 Bass kernel for nn_CrossAttention (linear-attention block).

Math (per batch b):
    q = x @ Wq; k = x @ Wk; v = x @ Wv
    q_sm = softmax(q, axis=-1) * D^-0.5          (feature softmax)
    k_sm = softmax(k, axis=-2)                   (sequence softmax)
    ctx  = k_sm^T @ v                            [D, D]
    out  = (q_sm @ ctx) @ Wo + bo
    y    = layernorm(x + out) * gamma + beta

Sharding: data-parallel, one batch per NeuronCore (B == 8 == n_cores).

Device algorithm (no softmax max-subtraction needed: q,k ~ N(0,1)):
    ek = exp(k);  S_k[d] = sum_l ek[l,d]
    CT[e,d]   = sum_l v[l,e] * ek[l,d]           (ctx^T, unnormalized)
    C2[d,e']  = (CT^T @ Wo)[d,e'] * D^-0.5 / S_k[d]
    eq = exp(q);  S_q[l] = sum_d eq[l,d]
    out[l,e'] = (eq @ C2)[l,e'] / S_q[l]
    y = LN(x + bo + out)
All matmuls in bf16 with fp32 PSUM accumulation. x^T (bf16) is the only
stationary source for the projections; q is produced transposed (qT) so
exp(qT) blocks serve directly as matmul lhsT for the attn GEMM.
"""

import numpy as np
import ml_dtypes
from contextlib import ExitStack

B, L, D = 8, 4096, 1024
P = 128
LT = L // P       # 32 row tiles
FC = D // P       # 8 feature chunks
NG = 8            # l-groups for the q phase
GL = L // NG      # 512 columns per l-group
LN_EPS = 1e-5

_CACHE = {}


def _build_nc():
    import concourse.tile as tile
    from concourse import bacc, mybir

    f32 = mybir.dt.float32
    bf16 = mybir.dt.bfloat16
    FT = mybir.ActivationFunctionType
    OP = mybir.AluOpType

    nc = bacc.Bacc("TRN2", target_bir_lowering=False, debug=False,
                   enable_asserts=False)

    xT = nc.dram_tensor("xT", [D, L], bf16, kind="ExternalInput").ap()
    xr = nc.dram_tensor("xr", [L, D], f32, kind="ExternalInput").ap()
    wk_d = nc.dram_tensor("wk", [D, D], bf16, kind="ExternalInput").ap()
    wv_d = nc.dram_tensor("wv", [D, D], bf16, kind="ExternalInput").ap()
    wq_d = nc.dram_tensor("wq", [D, D], bf16, kind="ExternalInput").ap()
    wo_d = nc.dram_tensor("wo", [D, D], bf16, kind="ExternalInput").ap()
    gam_d = nc.dram_tensor("gamma_bc", [P, D], f32, kind="ExternalInput").ap()
    bet_d = nc.dram_tensor("beta_bc", [P, D], f32, kind="ExternalInput").ap()
    ones_d = nc.dram_tensor("ones_col", [P, 1], bf16, kind="ExternalInput").ap()
    y_d = nc.dram_tensor("y", [L, D], f32, kind="ExternalOutput").ap()

    with tile.TileContext(nc) as tc, ExitStack() as ctx:
        wpool = ctx.enter_context(tc.tile_pool(name="w", bufs=2))
        ekp = ctx.enter_context(tc.tile_pool(name="ekp", bufs=32))
        sbp = ctx.enter_context(tc.tile_pool(name="sbp", bufs=2))
        eqp = ctx.enter_context(tc.tile_pool(name="eqp", bufs=10))
        xtgp = ctx.enter_context(tc.tile_pool(name="xtgp", bufs=12))
        ctp = ctx.enter_context(tc.tile_pool(name="ctp", bufs=8))
        c2p = ctx.enter_context(tc.tile_pool(name="c2p", bufs=8))
        cst = ctx.enter_context(tc.tile_pool(name="cst", bufs=1))
        vec = ctx.enter_context(tc.tile_pool(name="vec", bufs=2))
        ps_big = ctx.enter_context(tc.tile_pool(name="psb", bufs=2, space="PSUM"))
        ps_qt = ctx.enter_context(tc.tile_pool(name="psq", bufs=2, space="PSUM"))
        ps_sk = ctx.enter_context(tc.tile_pool(name="pssk", bufs=1, space="PSUM"))
        dram = ctx.enter_context(tc.tile_pool(name="dram", bufs=1, space="DRAM"))

        # ---- constants / weights -------------------------------------
        # Weights are DMA'd per f-chunk so the first projection matmul only
        # waits on chunk 0, not the whole 2 MB tensor.
        def load_w(dram_ap, name):
            t = wpool.tile([P, FC, D], bf16, tag="W", name=name)
            wr = dram_ap.rearrange("(fo fi) d -> fi fo d", fi=P)
            for f in range(FC):
                nc.sync.dma_start(t[:, f, :], wr[:, f, :])
            return t

        ones_sb = cst.tile([P, 1], bf16, tag="ones")
        nc.sync.dma_start(ones_sb[:], ones_d[:])
        eps_sb = cst.tile([P, 1], f32, tag="eps")
        nc.vector.memset(eps_sb[:], LN_EPS)

        v_spill = dram.tile([L, D], bf16)
        sk_dram = dram.tile([1, D], f32)

        xTr = xT.rearrange("(fo fi) l -> fi fo l", fi=P)

        # First x^T tile ahead of the weight DMAs: the startup critical path
        # is xt_0 + wk chunk 0.
        xt_0 = sbp.tile([P, FC, P], bf16, tag="xta", name="xt_0")
        for f in range(FC):
            nc.sync.dma_start(xt_0[:, f, :], xTr[:, f, 0:P])

        wk = load_w(wk_d, "wk")
        wv = load_w(wv_d, "wv")

        # ---- phase A: k/v projections, exp(k), S_k, spill v ----------
        ek_tiles = []
        sk_ps = ps_sk.tile([1, D], f32, tag="skq")
        for i in range(LT):
            if i == 0:
                xt_i = xt_0
            else:
                xt_i = sbp.tile([P, FC, P], bf16, tag="xta", name=f"xt_{i}")
                nc.sync.dma_start(xt_i[:], xTr[:, :, i * P:(i + 1) * P])

            k_ps = ps_big.tile([P, D], f32, tag="big")
            for f in range(FC):
                for n0 in (0, 512):
                    nc.tensor.matmul(k_ps[:, n0:n0 + 512], xt_i[:, f, :],
                                     wk[:, f, n0:n0 + 512],
                                     start=(f == 0), stop=(f == FC - 1))
            ek_i = ekp.tile([P, D], bf16, tag="ek")
            nc.scalar.activation(ek_i[:], k_ps[:], FT.Exp)
            ek_tiles.append(ek_i)

            v_ps = ps_big.tile([P, D], f32, tag="big")
            for f in range(FC):
                for n0 in (0, 512):
                    nc.tensor.matmul(v_ps[:, n0:n0 + 512], xt_i[:, f, :],
                                     wv[:, f, n0:n0 + 512],
                                     start=(f == 0), stop=(f == FC - 1))
            v_st = sbp.tile([P, D], bf16, tag="vst")
            nc.vector.tensor_copy(v_st[:], v_ps[:])
            nc.sync.dma_start(v_spill[i * P:(i + 1) * P, :], v_st[:])

            # S_k[d] += sum_{l in tile} ek[l, d]
            for n0 in (0, 512):
                nc.tensor.matmul(sk_ps[:, n0:n0 + 512], ones_sb[:],
                                 ek_i[:, n0:n0 + 512],
                                 start=(i == 0), stop=(i == LT - 1))

        # S_k row -> [P, FC] column layout via DRAM round-trip
        sk_row = cst.tile([1, D], f32, tag="skrow")
        nc.vector.tensor_copy(sk_row[:], sk_ps[:])
        nc.sync.dma_start(sk_dram[:], sk_row[:])
        sk_col = cst.tile([P, FC], f32, tag="skcol")
        nc.sync.dma_start(sk_col[:], sk_dram[0, :].rearrange("(o p) -> p o", p=P))
        rsk = cst.tile([P, FC], f32, tag="rsk")
        nc.vector.reciprocal(rsk[:], sk_col[:])
        nc.vector.tensor_scalar_mul(rsk[:], rsk[:], float(D) ** -0.5)

        # ---- context: CT[e,d] = sum_l v[l,e] * ek[l,d] ---------------
        ct_tiles = []
        for e in range(FC):
            c_ps = ps_big.tile([P, D], f32, tag="big")
            for j in range(4):
                vblk = sbp.tile([P, 8, P], bf16, tag="vct")
                nc.sync.dma_start(
                    vblk[:],
                    v_spill[j * 1024:(j + 1) * 1024, e * P:(e + 1) * P]
                    .rearrange("(o p) c -> p o c", p=P))
                for l8 in range(8):
                    lc = j * 8 + l8
                    for n0 in (0, 512):
                        nc.tensor.matmul(c_ps[:, n0:n0 + 512], vblk[:, l8, :],
                                         ek_tiles[lc][:, n0:n0 + 512],
                                         start=(lc == 0), stop=(lc == LT - 1))
            ct_e = ctp.tile([P, D], bf16, tag="ct")
            nc.vector.tensor_copy(ct_e[:], c_ps[:])
            ct_tiles.append(ct_e)

        wq = load_w(wq_d, "wq")
        wo = load_w(wo_d, "wo")
        gam_sb = cst.tile([P, D], f32, tag="gam")
        nc.sync.dma_start(gam_sb[:], gam_d[:])
        bet_sb = cst.tile([P, D], f32, tag="bet")
        nc.sync.dma_start(bet_sb[:], bet_d[:])

        # ---- C2[d, e'] = (CT^T @ Wo) * rsk[d] ------------------------
        c2_tiles = []
        for d in range(FC):
            c2_ps = ps_big.tile([P, D], f32, tag="big")
            for ec in range(FC):
                for n0 in (0, 512):
                    nc.tensor.matmul(c2_ps[:, n0:n0 + 512],
                                     ct_tiles[ec][:, d * P:(d + 1) * P],
                                     wo[:, ec, n0:n0 + 512],
                                     start=(ec == 0), stop=(ec == FC - 1))
            c2_d = c2p.tile([P, D], bf16, tag="c2")
            nc.vector.tensor_scalar_mul(c2_d[:], c2_ps[:], rsk[:, d:d + 1])
            c2_tiles.append(c2_d)

        # ---- phase B: qT, exp, attn, residual + layernorm ------------
        for g in range(NG):
            xtg = [xtgp.tile([P, GL], bf16, tag="xtg", name=f"xtg_{g}_{f}")
                   for f in range(FC)]
            for f in range(FC):
                nc.sync.dma_start(xtg[f][:], xTr[:, f, g * GL:(g + 1) * GL])

            eq_g = []
            for d in range(FC):
                qt_ps = ps_qt.tile([P, GL], f32, tag="qt")
                for f in range(FC):
                    nc.tensor.matmul(qt_ps[:], wq[:, f, d * P:(d + 1) * P],
                                     xtg[f][:],
                                     start=(f == 0), stop=(f == FC - 1))
                eq_d = eqp.tile([P, GL], bf16, tag="eq")
                nc.scalar.activation(eq_d[:], qt_ps[:], FT.Exp)
                eq_g.append(eq_d)

            for t in range(GL // P):
                lt = g * (GL // P) + t
                ts = slice(t * P, (t + 1) * P)
                at_ps = ps_big.tile([P, D], f32, tag="big")
                for d in range(FC):
                    for n0 in (0, 512):
                        nc.tensor.matmul(at_ps[:, n0:n0 + 512],
                                         eq_g[d][:, ts],
                                         c2_tiles[d][:, n0:n0 + 512],
                                         start=(d == 0), stop=(d == FC - 1))
                sq_ps = ps_sk.tile([P, 1], f32, tag="skq", name=f"sq_{lt}")
                for d in range(FC):
                    nc.tensor.matmul(sq_ps[:], eq_g[d][:, ts], ones_sb[:],
                                     start=(d == 0), stop=(d == FC - 1))
                s_q = vec.tile([P, 1], f32, tag="s_q")
                nc.vector.reciprocal(s_q[:], sq_ps[:])

                xr_t = sbp.tile([P, D], f32, tag="xr")
                nc.sync.dma_start(xr_t[:], xr[lt * P:(lt + 1) * P, :])

                r_t = sbp.tile([P, D], f32, tag="r")
                rsum = vec.tile([P, 1], f32, tag="rsum")
                nc.vector.scalar_tensor_tensor(
                    r_t[:], at_ps[:], s_q[:], xr_t[:],
                    op0=OP.mult, op1=OP.add, accum_out=rsum[:])

                sq_scr = ps_big.tile([P, D], f32, tag="big")
                rsumsq = vec.tile([P, 1], f32, tag="rsumsq")
                nc.scalar.activation(sq_scr[:], r_t[:], FT.Square,
                                     accum_out=rsumsq[:])

                mu = vec.tile([P, 1], f32, tag="mu")
                nc.vector.tensor_scalar_mul(mu[:], rsum[:], 1.0 / D)
                musq = vec.tile([P, 1], f32, tag="musq")
                nc.vector.tensor_mul(musq[:], mu[:], mu[:])
                var = vec.tile([P, 1], f32, tag="var")
                nc.vector.scalar_tensor_tensor(
                    var[:], rsumsq[:], 1.0 / D, musq[:],
                    op0=OP.mult, op1=OP.subtract)
                std = vec.tile([P, 1], f32, tag="std")
                nc.scalar.activation(std[:], var[:], FT.Sqrt, bias=eps_sb[:])
                rstd = vec.tile([P, 1], f32, tag="rstd")
                nc.vector.reciprocal(rstd[:], std[:])

                nc.vector.scalar_tensor_tensor(
                    r_t[:], r_t[:], mu[:], gam_sb[:],
                    op0=OP.subtract, op1=OP.mult)
                nc.vector.scalar_tensor_tensor(
                    r_t[:], r_t[:], rstd[:], bet_sb[:],
                    op0=OP.mult, op1=OP.add)
                nc.sync.dma_start(y_d[lt * P:(lt + 1) * P, :], r_t[:])

    nc.compile()
    return nc


def _get_nc():
    if "nc" not in _CACHE:
        _CACHE["nc"] = _build_nc()
    return _CACHE["nc"]


def make_in_maps(x, Wq, Wk, Wv, Wo, bo, gamma, beta):
    bf = ml_dtypes.bfloat16
    x = np.asarray(x, dtype=np.float32)
    in_maps = []
    wk_b = np.asarray(Wk, np.float32).astype(bf)
    wv_b = np.asarray(Wv, np.float32).astype(bf)
    wq_b = np.asarray(Wq, np.float32).astype(bf)
    wo_b = np.asarray(Wo, np.float32).astype(bf)
    gam = np.ascontiguousarray(
        np.broadcast_to(np.asarray(gamma, np.float32), (P, D)))
    bet = np.ascontiguousarray(
        np.broadcast_to(np.asarray(beta, np.float32), (P, D)))
    ones = np.ones((P, 1), dtype=bf)
    bo32 = np.asarray(bo, np.float32)
    for b in range(B):
        in_maps.append({
            "xT": np.ascontiguousarray(x[b].T).astype(bf),
            "xr": x[b] + bo32,
            "wk": wk_b, "wv": wv_b, "wq": wq_b, "wo": wo_b,
            "gamma_bc": gam, "beta_bc": bet, "ones_col": ones,
        })
    return in_maps


def kernel(x, Wq, Wk, Wv, Wo, bo, gamma, beta):
    from concourse import bass_utils
    nc = _get_nc()
    in_maps = make_in_maps(x, Wq, Wk, Wv, Wo, bo, gamma, beta)
    res = bass_utils.run_bass_kernel_spmd(nc, in_maps, core_ids=list(range(B)))
    out = np.stack([res.results[b]["y"] for b in range(B)], axis=0)
    return out.astype(np.float32)
